# revision 1
# baseline (speedup 1.0000x reference)
"""DCNv4 block (conv1x1+BN+SiLU -> value/offset proj -> deformable agg -> out proj+BN+SiLU)
on 8 trn2 NeuronCores. Data-parallel over (sample, row-half) with 3/4-row halos.

Deformable aggregation strategy: all 36 bilinear corners per (token, group) land in a
fixed 8x7 patch around the token (offsets are small). Patch weights are built densely
with hat functions (no floor/gather), scattered into a dense sparse-matrix row block
S^T[token, (rho, w')] via gpsimd local_scatter with a constant shear index table,
DMA-transposed to S[(w'), rho, token], and contracted against token-major values on
the PE: dcn^T[c, t] = sum_rho v^T[w', row, c]^T @ S[w', rho, t].
"""
import numpy as np

from concourse import bass, mybir, tile, bacc, bass_utils

# ---- problem constants (hardcoded; kernel.py must be self-contained) ----
N, C, H, W = 4, 256, 128, 128
G, KS, K = 4, 3, 9
Cg = C // G
PAD_OFF = 112
EPS = 1e-5
NCORES = 8
HS = H // 2                    # interior rows per core
RV = 72                        # v rows per core: 3 halo top + 64 + 4 halo bottom + 1 pad
RHO, DEL = 8, 7                # patch extent (rows x cols)
NSLOT = RHO * DEL              # 56
TAU = RHO * W                  # 1024
NBLK = RV // 4                 # stage-1/2 row blocks of 4

fp32 = mybir.dt.float32
fp16 = mybir.dt.float16
i16 = mybir.dt.int16
AF = mybir.ActivationFunctionType
ALU = mybir.AluOpType


def _emit(tc, nc, io):
    P = 128
    x_sh, cw, bn1s, bn1b, wvo, brow, ones1, kyc, kxc, sidx, owT, bn2s, bn2b, rowmask, out_d = io

    with tc.tile_pool(name="const", bufs=1) as cp, \
         tc.tile_pool(name="big", bufs=1) as bp, \
         tc.tile_pool(name="s12", bufs=2) as p12, \
         tc.tile_pool(name="s12ps", bufs=2, space="PSUM") as ps12, \
         tc.tile_pool(name="s3", bufs=2) as p3, \
         tc.tile_pool(name="s3ps", bufs=2, space="PSUM") as ps3:

        # ---- load constants ----
        cw_sb = cp.tile([P, 2, 256], fp32)
        wvo_sb = cp.tile([P, 2, 368], fp32)
        brow_sb = cp.tile([1, 368], fp32)
        ones_sb = cp.tile([1, P], fp32)
        bn1s_sb = cp.tile([P, 2], fp32)
        bn1b_sb = cp.tile([P, 2], fp32)
        kyc_sb = cp.tile([P, 36, RHO], fp32)
        kxc_sb = cp.tile([P, 36, DEL], fp32)
        sidx_sb = cp.tile([P, NSLOT], i16)
        owT_sb = cp.tile([P, 2, 2, P], fp16)
        bn2s_sb = cp.tile([P, 2], fp32)
        bn2b_sb = cp.tile([P, 2], fp32)
        rmask_sb = cp.tile([P, RV], fp16)
        for sb, dr in ((cw_sb, cw), (wvo_sb, wvo), (brow_sb, brow), (ones_sb, ones1),
                       (bn1s_sb, bn1s), (bn1b_sb, bn1b), (kyc_sb, kyc), (kxc_sb, kxc),
                       (sidx_sb, sidx), (owT_sb, owT), (bn2s_sb, bn2s), (bn2b_sb, bn2b),
                       (rmask_sb, rowmask)):
            nc.sync.dma_start(sb[:], dr)

        v_sb = bp.tile([P, RV, 256], fp16)
        om_sb = bp.tile([P, HS, 108], fp32)

        # ================= stage 1+2: conv+BN+SiLU, value/offset proj =================
        for blk in range(NBLK):
            x_t = p12.tile([P, 2, 512], fp32, tag="x")
            for ci in range(2):
                nc.sync.dma_start(x_t[:, ci, :], x_sh[ci, :, blk * 512:(blk + 1) * 512])
            y_sb = p12.tile([P, 2, 512], fp32, tag="y")
            for co in range(2):
                y_ps = ps12.tile([P, 512], fp32, space="PSUM", tag="yps")
                for ci in range(2):
                    nc.tensor.matmul(out=y_ps[:], lhsT=cw_sb[:, ci, co * P:(co + 1) * P],
                                     rhs=x_t[:, ci, :], start=(ci == 0), stop=(ci == 1))
                nc.scalar.activation(y_sb[:, co, :], y_ps[:], AF.Silu,
                                     scale=bn1s_sb[:, co:co + 1], bias=bn1b_sb[:, co:co + 1])
            for r4 in range(4):
                rr = blk * 4 + r4
                p_ps = ps12.tile([P, 368], fp32, space="PSUM", tag="pps")
                for ci in range(2):
                    nc.tensor.matmul(out=p_ps[:], lhsT=y_sb[:, ci, r4 * P:(r4 + 1) * P],
                                     rhs=wvo_sb[:, ci, :], start=(ci == 0), stop=False)
                nc.tensor.matmul(out=p_ps[:], lhsT=ones_sb[:], rhs=brow_sb[:],
                                 start=False, stop=True)
                nc.scalar.activation(v_sb[:, rr, :], p_ps[:, 0:256], AF.Copy)
                if 3 <= rr < 3 + HS:
                    nc.scalar.activation(om_sb[:, rr - 3, :], p_ps[:, 256:364], AF.Copy)

        # zero out-of-image halo rows of v (per-core row mask)
        nc.vector.tensor_tensor(out=v_sb[:], in0=v_sb[:],
                                in1=rmask_sb[:].unsqueeze(2).to_broadcast([P, RV, 256]),
                                op=ALU.mult)

        # ================= stage 3: deformable aggregation per output row =============
        for h in range(HS):
            offy = om_sb[:, h, 0:36]
            offx = om_sb[:, h, 36:72]
            msk = om_sb[:, h, 72:108]

            uy = p3.tile([P, 36, RHO], fp32, tag="uy")
            nc.vector.tensor_tensor(out=uy[:], in0=kyc_sb[:],
                                    in1=offy.unsqueeze(2).to_broadcast([P, 36, RHO]),
                                    op=ALU.subtract)
            nc.scalar.activation(uy[:], uy[:], AF.Abs)
            nc.scalar.activation(uy[:], uy[:], AF.Relu, scale=-1.0, bias=1.0)
            aym = p3.tile([P, 36, RHO], fp32, tag="aym")
            nc.vector.tensor_tensor(out=aym[:], in0=uy[:],
                                    in1=msk.unsqueeze(2).to_broadcast([P, 36, RHO]),
                                    op=ALU.mult)
            ux = p3.tile([P, 36, DEL], fp32, tag="ux")
            nc.vector.tensor_tensor(out=ux[:], in0=kxc_sb[:],
                                    in1=offx.unsqueeze(2).to_broadcast([P, 36, DEL]),
                                    op=ALU.subtract)
            nc.scalar.activation(ux[:], ux[:], AF.Abs)
            nc.scalar.activation(ux[:], ux[:], AF.Relu, scale=-1.0, bias=1.0)

            # prod memory layout [g][rho][del][k]; write iterated as (g,k,rho,del)
            prod = p3.tile([P, G, RHO, DEL, K], fp32, tag="prod")
            P16 = p3.tile([P, G, NSLOT], fp16, tag="P16")
            for g in range(G):
                pv = prod[:, g].rearrange("p r d k -> p k r d")
                nc.vector.tensor_tensor(
                    out=pv,
                    in0=aym[:, g * K:(g + 1) * K, :].unsqueeze(3).to_broadcast([P, K, RHO, DEL]),
                    in1=ux[:, g * K:(g + 1) * K, :].unsqueeze(2).to_broadcast([P, K, RHO, DEL]),
                    op=ALU.mult)
                P32g = p3.tile([P, NSLOT], fp32, tag="P32g")
                nc.vector.tensor_reduce(out=P32g[:],
                                        in_=prod[:, g].rearrange("p r d k -> p (r d) k"),
                                        axis=mybir.AxisListType.X, op=ALU.add)
                nc.vector.tensor_copy(out=P16[:, g, :], in_=P32g[:])

            dc = ps3.tile([P, 2, P], fp32, space="PSUM", tag="dc")
            for g in range(G):
                ST = p3.tile([P, TAU], fp16, tag=f"ST{g}")
                nc.gpsimd.local_scatter(ST[:], P16[:, g, :], sidx_sb[:],
                                        channels=P, num_elems=TAU, num_idxs=NSLOT)
                S = p3.tile([W, RHO, P], fp16, tag=f"S{g}")
                nc.sync.dma_start_transpose(out=S[:], in_=ST[:])
                po = (g % 2) * 64
                for rho in range(RHO):
                    nc.tensor.matmul(out=dc[po:po + 64, g // 2, :],
                                     lhsT=v_sb[:, h + rho, g * Cg:(g + 1) * Cg],
                                     rhs=S[:, rho, :], start=(rho == 0), stop=(rho == 7))
            dcn = p3.tile([P, 2, P], fp16, tag="dcn")
            for half in range(2):
                nc.scalar.activation(dcn[:, half, :], dc[:, half, :], AF.Copy)

            o_ps = ps3.tile([P, 2, P], fp32, space="PSUM", tag="ops")
            for co in range(2):
                for ci in range(2):
                    nc.tensor.matmul(out=o_ps[:, co, :], lhsT=owT_sb[:, ci, co, :],
                                     rhs=dcn[:, ci, :], start=(ci == 0), stop=(ci == 1))
            out_sb = p3.tile([P, 2, P], fp32, tag="osb")
            for co in range(2):
                nc.scalar.activation(out_sb[:, co, :], o_ps[:, co, :], AF.Silu,
                                     scale=bn2s_sb[:, co:co + 1], bias=bn2b_sb[:, co:co + 1])
                nc.sync.dma_start(out_d[co, :, h * P:(h + 1) * P], out_sb[:, co, :])


_CACHE = {}


def _build():
    if "nc" in _CACHE:
        return _CACHE["nc"], _CACHE["io_names"]
    nc = bacc.Bacc("TRN2", target_bir_lowering=False, debug=False, num_devices=NCORES)
    P = 128
    specs = [
        ("x_sh", [2, P, RV * W], fp32, "ExternalInput"),
        ("cw", [P, 2, 256], fp32, "ExternalInput"),
        ("bn1s", [P, 2], fp32, "ExternalInput"),
        ("bn1b", [P, 2], fp32, "ExternalInput"),
        ("wvo", [P, 2, 368], fp32, "ExternalInput"),
        ("brow", [1, 368], fp32, "ExternalInput"),
        ("ones1", [1, P], fp32, "ExternalInput"),
        ("kyc", [P, 36, RHO], fp32, "ExternalInput"),
        ("kxc", [P, 36, DEL], fp32, "ExternalInput"),
        ("sidx", [P, NSLOT], i16, "ExternalInput"),
        ("owT", [P, 2, 2, P], fp16, "ExternalInput"),
        ("bn2s", [P, 2], fp32, "ExternalInput"),
        ("bn2b", [P, 2], fp32, "ExternalInput"),
        ("rowmask", [P, RV], fp16, "ExternalInput"),
        ("out", [2, P, HS * W], fp32, "ExternalOutput"),
    ]
    io = [nc.dram_tensor(nm, sh, dt, kind=kd).ap() for nm, sh, dt, kd in specs]
    with tile.TileContext(nc) as tc:
        _emit(tc, nc, io)
    nc.compile()
    _CACHE["nc"] = nc
    _CACHE["io_names"] = [s[0] for s in specs]
    return nc, _CACHE["io_names"]


def _host_prep(inputs):
    """Build the shared (weights/consts) and per-core input arrays."""
    P = 128
    f32 = np.float32
    conv_w = np.asarray(inputs["conv_w"], f32)[:, :, 0, 0]       # [co, ci]
    value_w = np.asarray(inputs["value_w"], f32)                  # [co, ci]
    offset_w = np.asarray(inputs["offset_w"], f32)                # [112, ci]
    out_w = np.asarray(inputs["out_w"], f32)                      # [co, ci]

    cw = conv_w.T.reshape(2, P, 256).transpose(1, 0, 2).astype(f32).copy()                  # [ci_chunk, ci_p, co]
    s1 = (np.asarray(inputs["bn1_gamma"], f32)
          / np.sqrt(np.asarray(inputs["bn1_var"], f32) + EPS))
    b1 = np.asarray(inputs["bn1_beta"], f32) - np.asarray(inputs["bn1_mean"], f32) * s1
    bn1s = s1.reshape(2, P).T.copy()                              # [p, co_chunk]
    bn1b = b1.reshape(2, P).T.copy()

    # permuted offset rows: [y(g,k) 36 | x(g,k) 36 | mask(g,k) 36]
    perm = np.empty(108, np.int64)
    for g in range(G):
        for k in range(K):
            perm[g * K + k] = g * 27 + 2 * k + 1
            perm[36 + g * K + k] = g * 27 + 2 * k
            perm[72 + g * K + k] = g * 27 + 18 + k
    ow_p = offset_w[perm]                                         # [108, ci]
    ob_p = np.asarray(inputs["offset_b"], f32)[perm]
    wvo_full = np.concatenate([value_w.T, ow_p.T, np.zeros((256, 4), f32)], axis=1)
    wvo = wvo_full.reshape(2, P, 368).transpose(1, 0, 2).astype(f32).copy()
    brow = np.concatenate([np.asarray(inputs["value_b"], f32), ob_p,
                           np.zeros(4, f32)]).reshape(1, 368)
    ones1 = np.ones((1, P), f32)

    ks = np.arange(K)
    ik, jk = ks // 3, ks % 3
    rho = np.arange(RHO)
    dl = np.arange(DEL)
    kyc1 = rho[None, :] - 3 - (ik[:, None] - 1)                   # [k, rho]
    kxc1 = dl[None, :] - 3 - (jk[:, None] - 1)                    # [k, del]
    kyc = np.broadcast_to(np.tile(kyc1, (G, 1)).reshape(1, 36, RHO),
                          (P, 36, RHO)).astype(f32).copy()
    kxc = np.broadcast_to(np.tile(kxc1, (G, 1)).reshape(1, 36, DEL),
                          (P, 36, DEL)).astype(f32).copy()

    sidx = np.empty((P, NSLOT), np.int16)
    for t in range(P):
        for r in range(RHO):
            for d in range(DEL):
                w = t + d - 3
                sidx[t, r * DEL + d] = r * W + w if 0 <= w < W else -1

    owT = np.empty((P, 2, 2, P), np.float16)
    for ci in range(2):
        for co in range(2):
            owT[:, ci, co, :] = out_w[co * P:(co + 1) * P, ci * P:(ci + 1) * P].T
    s2 = (np.asarray(inputs["bn2_gamma"], f32)
          / np.sqrt(np.asarray(inputs["bn2_var"], f32) + EPS))
    b2 = np.asarray(inputs["bn2_beta"], f32) - np.asarray(inputs["bn2_mean"], f32) * s2
    bn2s = s2.reshape(2, P).T.copy()
    bn2b = b2.reshape(2, P).T.copy()

    shared = dict(cw=cw, bn1s=bn1s, bn1b=bn1b, wvo=wvo, brow=brow, ones1=ones1,
                  kyc=kyc, kxc=kxc, sidx=sidx, owT=owT, bn2s=bn2s, bn2b=bn2b)

    x = np.asarray(inputs["x"], f32)
    in_maps = []
    for c in range(NCORES):
        n, half = c // 2, c % 2
        h0 = half * HS
        lo, hi = h0 - 3, h0 + HS + 5                              # 72 rows
        xs = np.zeros((C, RV, W), f32)
        s, e = max(lo, 0), min(hi, H)
        xs[:, s - lo:e - lo, :] = x[n, :, s:e, :]
        rm = np.zeros((P, RV), np.float16)
        valid = np.zeros(RV, np.float16)
        valid[s - lo:e - lo] = 1.0
        rm[:] = valid[None, :]
        m = dict(shared)
        m["x_sh"] = xs.reshape(2, P, RV * W).astype(f32)
        m["rowmask"] = rm
        in_maps.append(m)
    return in_maps


def kernel(**inputs):
    nc, _ = _build()
    in_maps = _host_prep(inputs)
    res = bass_utils.run_bass_kernel_spmd(nc, in_maps, core_ids=list(range(NCORES)))
    out = np.empty((N, C, H, W), np.float32)
    for c in range(NCORES):
        n, half = c // 2, c % 2
        o = res.results[c]["out"]                                 # [2, 128, HS*W]
        for co in range(2):
            out[n, co * 128:(co + 1) * 128, half * HS:(half + 1) * HS, :] = \
                o[co].reshape(128, HS, W)
    return out



# revision 2
# speedup vs baseline: 85.5803x; 85.5803x over previous
"""DCNv4 block (conv1x1+BN+SiLU -> value/offset proj -> deformable agg -> out proj+BN+SiLU)
on 8 trn2 NeuronCores. Data-parallel over (sample, row-half) with 3/4-row halos.

Deformable aggregation strategy: all 36 bilinear corners per (token, group) land in a
fixed 8x7 patch around the token (offsets are small). Patch weights are built densely
with hat functions (no floor/gather), scattered into a dense sparse-matrix row block
S^T[token, (rho, w')] via gpsimd local_scatter with a constant shear index table,
DMA-transposed to S[(w'), rho, token], and contracted against token-major values on
the PE: dcn^T[c, t] = sum_rho v^T[w', row, c]^T @ S[w', rho, t].

Execution path: the axon tunnel (~20-40 MB/s) dominates end-to-end time, so the
runner (a) ships x and the matmul weights in fp16 and returns the output in fp16,
(b) builds the shard_map jit ONCE and reuses it (the stock run_bass_kernel_spmd
re-traces and re-compiles per call), (c) keeps the input-independent geometry
constants and the zero output operands resident on device (no donation -- this
kernel writes every output element, so fresh uninit result buffers are fine).
"""
import numpy as np

import jax
from jax.experimental.shard_map import shard_map
from jax.sharding import Mesh, NamedSharding, PartitionSpec

from concourse import bass, mybir, tile, bacc, bass_utils, bass2jax

# ---- problem constants (hardcoded; kernel.py must be self-contained) ----
N, C, H, W = 4, 256, 128, 128
G, KS, K = 4, 3, 9
Cg = C // G
PAD_OFF = 112
EPS = 1e-5
NCORES = 8
HS = H // 2                    # interior rows per core
RV = 72                        # v rows per core: 3 halo top + 64 + 4 halo bottom + 1 pad
RHO, DEL = 8, 7                # patch extent (rows x cols)
NSLOT = RHO * DEL              # 56
TAU = RHO * W                  # 1024
NBLK = RV // 4                 # stage-1/2 row blocks of 4

fp32 = mybir.dt.float32
fp16 = mybir.dt.float16
i16 = mybir.dt.int16
AF = mybir.ActivationFunctionType
ALU = mybir.AluOpType
P = 128

# input-independent inputs (geometry tables): uploaded once, cached on device
CONST_NAMES = ("ones1", "kyc", "kxc", "sidx", "rowmask")


def _emit(tc, nc, io):
    x_sh, cw, bn1s, bn1b, wvo, brow, ones1, kyc, kxc, sidx, owT, bn2s, bn2b, rowmask, out_d = io

    with tc.tile_pool(name="const", bufs=1) as cp, \
         tc.tile_pool(name="big", bufs=1) as bp, \
         tc.tile_pool(name="s12", bufs=2) as p12, \
         tc.tile_pool(name="s12ps", bufs=2, space="PSUM") as ps12, \
         tc.tile_pool(name="s3", bufs=2) as p3, \
         tc.tile_pool(name="s3ps", bufs=2, space="PSUM") as ps3:

        # ---- load constants ----
        cw_sb = cp.tile([P, 2, 256], fp16)
        wvo_sb = cp.tile([P, 2, 368], fp16)
        brow_sb = cp.tile([1, 368], fp16)
        ones_sb = cp.tile([1, P], fp16)
        bn1s_sb = cp.tile([P, 2], fp32)
        bn1b_sb = cp.tile([P, 2], fp32)
        kyc_sb = cp.tile([P, 36, RHO], fp32)
        kxc_sb = cp.tile([P, 36, DEL], fp32)
        sidx_sb = cp.tile([P, NSLOT], i16)
        owT_sb = cp.tile([P, 2, 2, P], fp16)
        bn2s_sb = cp.tile([P, 2], fp32)
        bn2b_sb = cp.tile([P, 2], fp32)
        rmask_sb = cp.tile([P, RV], fp16)
        for sb, dr in ((cw_sb, cw), (wvo_sb, wvo), (brow_sb, brow), (ones_sb, ones1),
                       (bn1s_sb, bn1s), (bn1b_sb, bn1b), (kyc_sb, kyc), (kxc_sb, kxc),
                       (sidx_sb, sidx), (owT_sb, owT), (bn2s_sb, bn2s), (bn2b_sb, bn2b),
                       (rmask_sb, rowmask)):
            nc.sync.dma_start(sb[:], dr)

        v_sb = bp.tile([P, RV, 256], fp16)
        om_sb = bp.tile([P, HS, 108], fp32)

        # ================= stage 1+2: conv+BN+SiLU, value/offset proj =================
        for blk in range(NBLK):
            x_t = p12.tile([P, 2, 512], fp16, tag="x")
            for ci in range(2):
                nc.sync.dma_start(x_t[:, ci, :], x_sh[ci, :, blk * 512:(blk + 1) * 512])
            y_sb = p12.tile([P, 2, 512], fp16, tag="y")
            for co in range(2):
                y_ps = ps12.tile([P, 512], fp32, space="PSUM", tag="yps")
                for ci in range(2):
                    nc.tensor.matmul(out=y_ps[:], lhsT=cw_sb[:, ci, co * P:(co + 1) * P],
                                     rhs=x_t[:, ci, :], start=(ci == 0), stop=(ci == 1))
                nc.scalar.activation(y_sb[:, co, :], y_ps[:], AF.Silu,
                                     scale=bn1s_sb[:, co:co + 1], bias=bn1b_sb[:, co:co + 1])
            for r4 in range(4):
                rr = blk * 4 + r4
                p_ps = ps12.tile([P, 368], fp32, space="PSUM", tag="pps")
                for ci in range(2):
                    nc.tensor.matmul(out=p_ps[:], lhsT=y_sb[:, ci, r4 * P:(r4 + 1) * P],
                                     rhs=wvo_sb[:, ci, :], start=(ci == 0), stop=False)
                nc.tensor.matmul(out=p_ps[:], lhsT=ones_sb[:], rhs=brow_sb[:],
                                 start=False, stop=True)
                nc.scalar.activation(v_sb[:, rr, :], p_ps[:, 0:256], AF.Copy)
                if 3 <= rr < 3 + HS:
                    nc.scalar.activation(om_sb[:, rr - 3, :], p_ps[:, 256:364], AF.Copy)

        # zero out-of-image halo rows of v (per-core row mask)
        nc.vector.tensor_tensor(out=v_sb[:], in0=v_sb[:],
                                in1=rmask_sb[:].unsqueeze(2).to_broadcast([P, RV, 256]),
                                op=ALU.mult)

        # ================= stage 3: deformable aggregation per output row =============
        for h in range(HS):
            offy = om_sb[:, h, 0:36]
            offx = om_sb[:, h, 36:72]
            msk = om_sb[:, h, 72:108]

            uy = p3.tile([P, 36, RHO], fp32, tag="uy")
            nc.vector.tensor_tensor(out=uy[:], in0=kyc_sb[:],
                                    in1=offy.unsqueeze(2).to_broadcast([P, 36, RHO]),
                                    op=ALU.subtract)
            nc.scalar.activation(uy[:], uy[:], AF.Abs)
            nc.scalar.activation(uy[:], uy[:], AF.Relu, scale=-1.0, bias=1.0)
            aym = p3.tile([P, 36, RHO], fp32, tag="aym")
            nc.vector.tensor_tensor(out=aym[:], in0=uy[:],
                                    in1=msk.unsqueeze(2).to_broadcast([P, 36, RHO]),
                                    op=ALU.mult)
            ux = p3.tile([P, 36, DEL], fp32, tag="ux")
            nc.vector.tensor_tensor(out=ux[:], in0=kxc_sb[:],
                                    in1=offx.unsqueeze(2).to_broadcast([P, 36, DEL]),
                                    op=ALU.subtract)
            nc.scalar.activation(ux[:], ux[:], AF.Abs)
            nc.scalar.activation(ux[:], ux[:], AF.Relu, scale=-1.0, bias=1.0)

            # prod memory layout [g][rho][del][k]; write iterated as (g,k,rho,del)
            prod = p3.tile([P, G, RHO, DEL, K], fp32, tag="prod")
            P16 = p3.tile([P, G, NSLOT], fp16, tag="P16")
            for g in range(G):
                pv = prod[:, g].rearrange("p r d k -> p k r d")
                nc.vector.tensor_tensor(
                    out=pv,
                    in0=aym[:, g * K:(g + 1) * K, :].unsqueeze(3).to_broadcast([P, K, RHO, DEL]),
                    in1=ux[:, g * K:(g + 1) * K, :].unsqueeze(2).to_broadcast([P, K, RHO, DEL]),
                    op=ALU.mult)
                P32g = p3.tile([P, NSLOT], fp32, tag="P32g")
                nc.vector.tensor_reduce(out=P32g[:],
                                        in_=prod[:, g].rearrange("p r d k -> p (r d) k"),
                                        axis=mybir.AxisListType.X, op=ALU.add)
                nc.vector.tensor_copy(out=P16[:, g, :], in_=P32g[:])

            dc = ps3.tile([P, 2, P], fp32, space="PSUM", tag="dc")
            for g in range(G):
                ST = p3.tile([P, TAU], fp16, tag=f"ST{g}")
                nc.gpsimd.local_scatter(ST[:], P16[:, g, :], sidx_sb[:],
                                        channels=P, num_elems=TAU, num_idxs=NSLOT)
                S = p3.tile([W, RHO, P], fp16, tag=f"S{g}")
                nc.sync.dma_start_transpose(out=S[:], in_=ST[:])
                po = (g % 2) * 64
                for rho in range(RHO):
                    nc.tensor.matmul(out=dc[po:po + 64, g // 2, :],
                                     lhsT=v_sb[:, h + rho, g * Cg:(g + 1) * Cg],
                                     rhs=S[:, rho, :], start=(rho == 0), stop=(rho == 7))
            dcn = p3.tile([P, 2, P], fp16, tag="dcn")
            for half in range(2):
                nc.scalar.activation(dcn[:, half, :], dc[:, half, :], AF.Copy)

            o_ps = ps3.tile([P, 2, P], fp32, space="PSUM", tag="ops")
            for co in range(2):
                for ci in range(2):
                    nc.tensor.matmul(out=o_ps[:, co, :], lhsT=owT_sb[:, ci, co, :],
                                     rhs=dcn[:, ci, :], start=(ci == 0), stop=(ci == 1))
            out_sb = p3.tile([P, 2, P], fp16, tag="osb")
            for co in range(2):
                nc.scalar.activation(out_sb[:, co, :], o_ps[:, co, :], AF.Silu,
                                     scale=bn2s_sb[:, co:co + 1], bias=bn2b_sb[:, co:co + 1])
                nc.sync.dma_start(out_d[co, :, h * P:(h + 1) * P], out_sb[:, co, :])


_CACHE = {}

SPECS = [
    ("x_sh", [2, P, RV * W], fp16, "ExternalInput"),
    ("cw", [P, 2, 256], fp16, "ExternalInput"),
    ("bn1s", [P, 2], fp32, "ExternalInput"),
    ("bn1b", [P, 2], fp32, "ExternalInput"),
    ("wvo", [P, 2, 368], fp16, "ExternalInput"),
    ("brow", [1, 368], fp16, "ExternalInput"),
    ("ones1", [1, P], fp16, "ExternalInput"),
    ("kyc", [P, 36, RHO], fp32, "ExternalInput"),
    ("kxc", [P, 36, DEL], fp32, "ExternalInput"),
    ("sidx", [P, NSLOT], i16, "ExternalInput"),
    ("owT", [P, 2, 2, P], fp16, "ExternalInput"),
    ("bn2s", [P, 2], fp32, "ExternalInput"),
    ("bn2b", [P, 2], fp32, "ExternalInput"),
    ("rowmask", [P, RV], fp16, "ExternalInput"),
    ("out", [2, P, HS * W], fp16, "ExternalOutput"),
]


def _const_globals():
    """Input-independent geometry tables, pre-concatenated over the 8 cores."""
    f32, f16 = np.float32, np.float16
    ks = np.arange(K)
    ik, jk = ks // 3, ks % 3
    rho = np.arange(RHO)
    dl = np.arange(DEL)
    kyc1 = rho[None, :] - 3 - (ik[:, None] - 1)                   # [k, rho]
    kxc1 = dl[None, :] - 3 - (jk[:, None] - 1)                    # [k, del]
    kyc = np.broadcast_to(np.tile(kyc1, (G, 1)).reshape(1, 36, RHO),
                          (P, 36, RHO)).astype(f32)
    kxc = np.broadcast_to(np.tile(kxc1, (G, 1)).reshape(1, 36, DEL),
                          (P, 36, DEL)).astype(f32)

    sidx = np.empty((P, NSLOT), np.int16)
    for t in range(P):
        for r in range(RHO):
            for d in range(DEL):
                w = t + d - 3
                sidx[t, r * DEL + d] = r * W + w if 0 <= w < W else -1

    rowmask = np.zeros((NCORES, P, RV), f16)
    for c in range(NCORES):
        half = c % 2
        h0 = half * HS
        lo, hi = h0 - 3, h0 + HS + 5
        s, e = max(lo, 0), min(hi, H)
        rowmask[c, :, s - lo:e - lo] = 1.0

    return {
        "ones1": np.ones((NCORES, P), f16),
        "kyc": np.tile(kyc, (NCORES, 1, 1)),
        "kxc": np.tile(kxc, (NCORES, 1, 1)),
        "sidx": np.tile(sidx, (NCORES, 1)),
        "rowmask": rowmask.reshape(NCORES * P, RV),
    }


def _build():
    if "fn" in _CACHE:
        return
    nc = bacc.Bacc("TRN2", target_bir_lowering=False, debug=False, num_devices=NCORES)
    io = [nc.dram_tensor(nm, sh, dt, kind=kd).ap() for nm, sh, dt, kd in SPECS]
    with tile.TileContext(nc) as tc:
        _emit(tc, nc, io)
    nc.compile()

    bass2jax.install_neuronx_cc_hook()
    partition_name = nc.partition_id_tensor.name if nc.partition_id_tensor else None
    in_names, out_names, out_avals = [], [], []
    for alloc in nc.m.functions[0].allocations:
        if not isinstance(alloc, mybir.MemoryLocationSet):
            continue
        name = alloc.memorylocations[0].name
        if alloc.kind == "ExternalInput":
            if name != partition_name:
                in_names.append(name)
        elif alloc.kind == "ExternalOutput":
            shape = tuple(alloc.tensor_shape)
            dtype = mybir.dt.np(alloc.dtype)
            out_names.append(name)
            out_avals.append(jax.core.ShapedArray(shape, dtype))
    assert nc.dbg_addr is None, "built with debug=False"
    n_params = len(in_names)
    all_names = list(in_names) + out_names
    if partition_name is not None:
        all_names.append(partition_name)

    def _body(*args):
        operands = list(args)
        if partition_name is not None:
            operands.append(bass2jax.partition_id_tensor())
        return tuple(bass2jax._bass_exec_p.bind(
            *operands,
            out_avals=tuple(out_avals),
            in_names=tuple(all_names),
            out_names=tuple(out_names),
            lowering_input_output_aliases=(),
            sim_require_finite=True,
            sim_require_nnan=True,
            nc=nc,
        ))

    devices = jax.devices()[:NCORES]
    mesh = Mesh(np.asarray(devices), ("core",))
    n_outs = len(out_names)
    fn = jax.jit(
        shard_map(_body, mesh=mesh,
                  in_specs=(PartitionSpec("core"),) * (n_params + n_outs),
                  out_specs=(PartitionSpec("core"),) * n_outs,
                  check_rep=False),
        keep_unused=True,
    )
    shd = NamedSharding(mesh, PartitionSpec("core"))
    # no donation: this kernel writes every output element, so the zero
    # "output operand" arrays are never consumed -- upload once, reuse forever
    zeros_host = [np.zeros((NCORES * a.shape[0], *a.shape[1:]), a.dtype) for a in out_avals]
    zeros_dev = [jax.device_put(z, shd) for z in zeros_host]
    consts_host = _const_globals()
    consts_dev = {k: jax.device_put(v, shd) for k, v in consts_host.items()}

    _CACHE.update(nc=nc, fn=fn, shd=shd, in_param_names=in_names,
                  out_names=out_names, out_avals=out_avals,
                  zeros_dev=zeros_dev, zeros_host=zeros_host,
                  consts_dev=consts_dev, consts_host=consts_host)


def _prep_globals(inputs):
    """Input-dependent global (8-core concatenated) arrays, keyed by tensor name."""
    f32, f16 = np.float32, np.float16
    conv_w = np.asarray(inputs["conv_w"], f32)[:, :, 0, 0]       # [co, ci]
    value_w = np.asarray(inputs["value_w"], f32)                  # [co, ci]
    offset_w = np.asarray(inputs["offset_w"], f32)                # [112, ci]
    out_w = np.asarray(inputs["out_w"], f32)                      # [co, ci]

    cw = conv_w.T.reshape(2, P, 256).transpose(1, 0, 2).astype(f16)  # [ci_p, ci_chunk, co]
    s1 = (np.asarray(inputs["bn1_gamma"], f32)
          / np.sqrt(np.asarray(inputs["bn1_var"], f32) + EPS))
    b1 = np.asarray(inputs["bn1_beta"], f32) - np.asarray(inputs["bn1_mean"], f32) * s1
    bn1s = s1.reshape(2, P).T.copy()                              # [p, co_chunk]
    bn1b = b1.reshape(2, P).T.copy()

    # permuted offset rows: [y(g,k) 36 | x(g,k) 36 | mask(g,k) 36]
    perm = np.empty(108, np.int64)
    for g in range(G):
        for k in range(K):
            perm[g * K + k] = g * 27 + 2 * k + 1
            perm[36 + g * K + k] = g * 27 + 2 * k
            perm[72 + g * K + k] = g * 27 + 18 + k
    ow_p = offset_w[perm]                                         # [108, ci]
    ob_p = np.asarray(inputs["offset_b"], f32)[perm]
    wvo_full = np.concatenate([value_w.T, ow_p.T, np.zeros((256, 4), f32)], axis=1)
    wvo = wvo_full.reshape(2, P, 368).transpose(1, 0, 2).astype(f16)
    brow = np.concatenate([np.asarray(inputs["value_b"], f32), ob_p,
                           np.zeros(4, f32)]).reshape(1, 368).astype(f16)

    owT = np.empty((P, 2, 2, P), f16)
    for ci in range(2):
        for co in range(2):
            owT[:, ci, co, :] = out_w[co * P:(co + 1) * P, ci * P:(ci + 1) * P].T
    s2 = (np.asarray(inputs["bn2_gamma"], f32)
          / np.sqrt(np.asarray(inputs["bn2_var"], f32) + EPS))
    b2 = np.asarray(inputs["bn2_beta"], f32) - np.asarray(inputs["bn2_mean"], f32) * s2
    bn2s = s2.reshape(2, P).T.copy()
    bn2b = b2.reshape(2, P).T.copy()

    x = np.asarray(inputs["x"], f32)
    xs = np.zeros((NCORES, 2, P, RV, W), f16)
    for c in range(NCORES):
        n, half = c // 2, c % 2
        h0 = half * HS
        lo, hi = h0 - 3, h0 + HS + 5                              # 72 rows
        s, e = max(lo, 0), min(hi, H)
        for ci in range(2):
            xs[c, ci, :, s - lo:e - lo, :] = x[n, ci * P:(ci + 1) * P, s:e, :]

    def rep(a):
        return np.broadcast_to(a[None], (NCORES, *a.shape)).reshape(
            NCORES * a.shape[0], *a.shape[1:])

    return {
        "x_sh": xs.reshape(NCORES * 2, P, RV * W),
        "cw": rep(cw), "bn1s": rep(bn1s), "bn1b": rep(bn1b),
        "wvo": rep(wvo), "brow": rep(brow), "owT": rep(owT),
        "bn2s": rep(bn2s), "bn2b": rep(bn2b),
    }


def _assemble(out_g):
    """(8*2, P, HS*W) fp16 -> (N, C, H, W) fp32."""
    g = np.asarray(out_g).reshape(N, 2, 2, P, HS, W)              # [n, half, co, p, h, w]
    return g.transpose(0, 2, 3, 1, 4, 5).reshape(N, C, H, W).astype(np.float32)


def _run_fast(glb):
    args = [_CACHE["consts_dev"][nm] if nm in _CACHE["consts_dev"]
            else jax.device_put(glb[nm], _CACHE["shd"])
            for nm in _CACHE["in_param_names"]]
    outs = _CACHE["fn"](*args, *_CACHE["zeros_dev"])
    return np.asarray(outs[0])


def _run_slow(glb):
    """Fallback: stock per-call runner."""
    full = dict(glb)
    full.update(_CACHE["consts_host"])
    in_maps = []
    for c in range(NCORES):
        m = {}
        for nm, sh, _, kd in SPECS:
            if kd != "ExternalInput":
                continue
            arr = full[nm]
            d0 = sh[0]
            m[nm] = np.ascontiguousarray(arr[c * d0:(c + 1) * d0])
        in_maps.append(m)
    res = bass_utils.run_bass_kernel_spmd(_CACHE["nc"], in_maps,
                                          core_ids=list(range(NCORES)))
    return np.concatenate([res.results[c]["out"] for c in range(NCORES)], axis=0)


def kernel(**inputs):
    _build()
    glb = _prep_globals(inputs)
    try:
        out_g = _run_fast(glb)
    except Exception:
        out_g = _run_slow(glb)
    return _assemble(out_g)


# revision 5
# speedup vs baseline: 2726.8667x; 31.8633x over previous
"""DCNv4 block (conv1x1+BN+SiLU -> value/offset proj -> deformable agg -> out proj+BN+SiLU)
on 8 trn2 NeuronCores. Data-parallel over (sample, row-half) with 3/4-row halos.

Deformable aggregation strategy: all 36 bilinear corners per (token, group) land in a
fixed 8x7 patch around the token (offsets are small). Patch weights are built densely
with hat functions (no floor/gather), scattered into a dense sparse-matrix row block
S^T[token, (rho, w')] via gpsimd local_scatter with a constant shear index table,
DMA-transposed to S[(w'), rho, token], and contracted against token-major values on
the PE: dcn^T[c, t] = sum_rho v^T[w', row, c]^T @ S[w', rho, t].

Execution path: the axon tunnel (~20-40 MB/s) dominates end-to-end time, so the
runner (a) ships x and the matmul weights in fp16 and returns the output in fp16,
(b) builds the shard_map jit ONCE and reuses it (the stock run_bass_kernel_spmd
re-traces and re-compiles per call), (c) keeps the input-independent geometry
constants and the zero output operands resident on device (no donation -- this
kernel writes every output element, so fresh uninit result buffers are fine).
"""
import numpy as np

import jax
from jax.experimental.shard_map import shard_map
from jax.sharding import Mesh, NamedSharding, PartitionSpec

from concourse import bass, mybir, tile, bacc, bass_utils, bass2jax

# ---- problem constants (hardcoded; kernel.py must be self-contained) ----
N, C, H, W = 4, 256, 128, 128
G, KS, K = 4, 3, 9
Cg = C // G
PAD_OFF = 112
EPS = 1e-5
NCORES = 8
HS = H // 2                    # interior rows per core
RV = 72                        # v rows per core: 3 halo top + 64 + 4 halo bottom + 1 pad
RHO, DEL = 8, 7                # patch extent (rows x cols)
NSLOT = RHO * DEL              # 56
TAU = RHO * W                  # 1024
NBLK = RV // 4                 # stage-1/2 row blocks of 4

fp32 = mybir.dt.float32
fp16 = mybir.dt.float16
i16 = mybir.dt.int16
AF = mybir.ActivationFunctionType
ALU = mybir.AluOpType
P = 128

# input-independent inputs (geometry tables): uploaded once, cached on device
CONST_NAMES = ("ones1", "kyc", "kxc", "sidx", "rowmask")


def _emit(tc, nc, io):
    x_sh, cw, bn1s, bn1b, wvo, brow, ones1, kyc, kxc, sidx, owT, bn2s, bn2b, rowmask, out_d = io

    with tc.tile_pool(name="const", bufs=1) as cp, \
         tc.tile_pool(name="big", bufs=1) as bp, \
         tc.tile_pool(name="s12", bufs=2) as p12, \
         tc.tile_pool(name="s12ps", bufs=2, space="PSUM") as ps12, \
         tc.tile_pool(name="s3", bufs=2) as p3, \
         tc.tile_pool(name="s3ps", bufs=2, space="PSUM") as ps3:

        # ---- load constants ----
        cw_sb = cp.tile([P, 2, 256], fp16)
        wvo_sb = cp.tile([P, 2, 368], fp16)
        brow_sb = cp.tile([1, 368], fp16)
        ones_sb = cp.tile([1, P], fp16)
        bn1s_sb = cp.tile([P, 2], fp32)
        bn1b_sb = cp.tile([P, 2], fp32)
        kyc_sb = cp.tile([P, 36, RHO], fp32)
        kxc_sb = cp.tile([P, 36, DEL], fp32)
        sidx_sb = cp.tile([P, NSLOT], i16)
        owT_sb = cp.tile([P, 2, 2, P], fp16)
        bn2s_sb = cp.tile([P, 2], fp32)
        bn2b_sb = cp.tile([P, 2], fp32)
        rmask_sb = cp.tile([P, RV], fp16)
        for sb, dr in ((cw_sb, cw), (wvo_sb, wvo), (brow_sb, brow), (ones_sb, ones1),
                       (bn1s_sb, bn1s), (bn1b_sb, bn1b), (kyc_sb, kyc), (kxc_sb, kxc),
                       (sidx_sb, sidx), (owT_sb, owT), (bn2s_sb, bn2s), (bn2b_sb, bn2b),
                       (rmask_sb, rowmask)):
            nc.sync.dma_start(sb[:], dr)

        v_sb = bp.tile([P, RV, 256], fp16)
        om_sb = bp.tile([P, HS, 108], fp32)
        x_sb = bp.tile([P, 2, RV * W], fp16)       # whole x shard SBUF-resident
        CH = 16                                     # output rows per staged DMA chunk
        out_c = bp.tile([P, 2, 2, CH * W], fp16)    # [p, slot, co, chunk]
        for ci in range(2):
            nc.sync.dma_start(x_sb[:, ci, :], x_sh[ci])

        # ================= stage 1+2: conv+BN+SiLU, value/offset proj =================
        for blk in range(NBLK):
            y_sb = p12.tile([P, 2, 512], fp16, tag="y")
            for co in range(2):
                y_ps = ps12.tile([P, 512], fp32, space="PSUM", tag="yps")
                for ci in range(2):
                    nc.tensor.matmul(out=y_ps[:], lhsT=cw_sb[:, ci, co * P:(co + 1) * P],
                                     rhs=x_sb[:, ci, blk * 512:(blk + 1) * 512],
                                     start=(ci == 0), stop=(ci == 1))
                nc.scalar.activation(y_sb[:, co, :], y_ps[:], AF.Silu,
                                     scale=bn1s_sb[:, co:co + 1], bias=bn1b_sb[:, co:co + 1])
            for r4 in range(4):
                rr = blk * 4 + r4
                p_ps = ps12.tile([P, 368], fp32, space="PSUM", tag="pps")
                for ci in range(2):
                    nc.tensor.matmul(out=p_ps[:], lhsT=y_sb[:, ci, r4 * P:(r4 + 1) * P],
                                     rhs=wvo_sb[:, ci, :], start=(ci == 0), stop=False)
                nc.tensor.matmul(out=p_ps[:], lhsT=ones_sb[:], rhs=brow_sb[:],
                                 start=False, stop=True)
                nc.scalar.activation(v_sb[:, rr, :], p_ps[:, 0:256], AF.Copy)
                if 3 <= rr < 3 + HS:
                    nc.scalar.activation(om_sb[:, rr - 3, :], p_ps[:, 256:364], AF.Copy)

        # zero out-of-image halo rows of v (per-core row mask)
        nc.vector.tensor_tensor(out=v_sb[:], in0=v_sb[:],
                                in1=rmask_sb[:].unsqueeze(2).to_broadcast([P, RV, 256]),
                                op=ALU.mult)

        # ================= stage 3: deformable aggregation per output row =============
        for h in range(HS):
            offy = om_sb[:, h, 0:36]
            offx = om_sb[:, h, 36:72]
            msk = om_sb[:, h, 72:108]

            uy = p3.tile([P, 36, RHO], fp32, tag="uy")
            nc.vector.tensor_tensor(out=uy[:], in0=kyc_sb[:],
                                    in1=offy.unsqueeze(2).to_broadcast([P, 36, RHO]),
                                    op=ALU.subtract)
            nc.scalar.activation(uy[:], uy[:], AF.Abs)
            nc.scalar.activation(uy[:], uy[:], AF.Relu, scale=-1.0, bias=1.0)
            aym = p3.tile([P, 36, RHO], fp32, tag="aym")
            nc.vector.tensor_tensor(out=aym[:], in0=uy[:],
                                    in1=msk.unsqueeze(2).to_broadcast([P, 36, RHO]),
                                    op=ALU.mult)
            ux = p3.tile([P, 36, DEL], fp32, tag="ux")
            nc.vector.tensor_tensor(out=ux[:], in0=kxc_sb[:],
                                    in1=offx.unsqueeze(2).to_broadcast([P, 36, DEL]),
                                    op=ALU.subtract)
            nc.scalar.activation(ux[:], ux[:], AF.Abs)
            nc.scalar.activation(ux[:], ux[:], AF.Relu, scale=-1.0, bias=1.0)

            # prod memory layout [g][rho][del][k]; write iterated as (g,k,rho,del)
            prod = p3.tile([P, G, RHO, DEL, K], fp32, tag="prod")
            P16 = p3.tile([P, G, NSLOT], fp16, tag="P16")
            for g in range(G):
                pv = prod[:, g].rearrange("p r d k -> p k r d")
                nc.vector.tensor_tensor(
                    out=pv,
                    in0=aym[:, g * K:(g + 1) * K, :].unsqueeze(3).to_broadcast([P, K, RHO, DEL]),
                    in1=ux[:, g * K:(g + 1) * K, :].unsqueeze(2).to_broadcast([P, K, RHO, DEL]),
                    op=ALU.mult)
                P32g = p3.tile([P, NSLOT], fp32, tag="P32g")
                nc.vector.tensor_reduce(out=P32g[:],
                                        in_=prod[:, g].rearrange("p r d k -> p (r d) k"),
                                        axis=mybir.AxisListType.X, op=ALU.add)
                nc.vector.tensor_copy(out=P16[:, g, :], in_=P32g[:])

            dc = ps3.tile([P, 2, P], fp32, space="PSUM", tag="dc")
            ST = p3.tile([P, G, TAU], fp16, tag="ST")
            for g in range(G):
                nc.gpsimd.local_scatter(ST[:, g, :], P16[:, g, :], sidx_sb[:],
                                        channels=P, num_elems=TAU, num_idxs=NSLOT)
            S = p3.tile([W, G, RHO, P], fp16, tag="S")
            nc.sync.dma_start_transpose(out=S[:], in_=ST[:])
            for g in range(G):
                po = (g % 2) * 64
                for rho in range(RHO):
                    nc.tensor.matmul(out=dc[po:po + 64, g // 2, :],
                                     lhsT=v_sb[:, h + rho, g * Cg:(g + 1) * Cg],
                                     rhs=S[:, g, rho, :], start=(rho == 0), stop=(rho == 7))
            dcn = p3.tile([P, 2, P], fp16, tag="dcn")
            for half in range(2):
                nc.scalar.activation(dcn[:, half, :], dc[:, half, :], AF.Copy)

            o_ps = ps3.tile([P, 2, P], fp32, space="PSUM", tag="ops")
            for co in range(2):
                for ci in range(2):
                    nc.tensor.matmul(out=o_ps[:, co, :], lhsT=owT_sb[:, ci, co, :],
                                     rhs=dcn[:, ci, :], start=(ci == 0), stop=(ci == 1))
            slot, hh = (h // CH) % 2, h % CH
            for co in range(2):
                nc.scalar.activation(out_c[:, slot, co, hh * W:(hh + 1) * W],
                                     o_ps[:, co, :], AF.Silu,
                                     scale=bn2s_sb[:, co:co + 1], bias=bn2b_sb[:, co:co + 1])
                if hh == CH - 1:
                    nc.sync.dma_start(out_d[co, :, (h - CH + 1) * W:(h + 1) * W],
                                      out_c[:, slot, co, :])


_CACHE = {}

SPECS = [
    ("x_sh", [2, P, RV * W], fp16, "ExternalInput"),
    ("cw", [P, 2, 256], fp16, "ExternalInput"),
    ("bn1s", [P, 2], fp32, "ExternalInput"),
    ("bn1b", [P, 2], fp32, "ExternalInput"),
    ("wvo", [P, 2, 368], fp16, "ExternalInput"),
    ("brow", [1, 368], fp16, "ExternalInput"),
    ("ones1", [1, P], fp16, "ExternalInput"),
    ("kyc", [P, 36, RHO], fp32, "ExternalInput"),
    ("kxc", [P, 36, DEL], fp32, "ExternalInput"),
    ("sidx", [P, NSLOT], i16, "ExternalInput"),
    ("owT", [P, 2, 2, P], fp16, "ExternalInput"),
    ("bn2s", [P, 2], fp32, "ExternalInput"),
    ("bn2b", [P, 2], fp32, "ExternalInput"),
    ("rowmask", [P, RV], fp16, "ExternalInput"),
    ("out", [2, P, HS * W], fp16, "ExternalOutput"),
]


def _const_globals():
    """Input-independent geometry tables, pre-concatenated over the 8 cores."""
    f32, f16 = np.float32, np.float16
    ks = np.arange(K)
    ik, jk = ks // 3, ks % 3
    rho = np.arange(RHO)
    dl = np.arange(DEL)
    kyc1 = rho[None, :] - 3 - (ik[:, None] - 1)                   # [k, rho]
    kxc1 = dl[None, :] - 3 - (jk[:, None] - 1)                    # [k, del]
    kyc = np.broadcast_to(np.tile(kyc1, (G, 1)).reshape(1, 36, RHO),
                          (P, 36, RHO)).astype(f32)
    kxc = np.broadcast_to(np.tile(kxc1, (G, 1)).reshape(1, 36, DEL),
                          (P, 36, DEL)).astype(f32)

    sidx = np.empty((P, NSLOT), np.int16)
    for t in range(P):
        for r in range(RHO):
            for d in range(DEL):
                w = t + d - 3
                sidx[t, r * DEL + d] = r * W + w if 0 <= w < W else -1

    rowmask = np.zeros((NCORES, P, RV), f16)
    for c in range(NCORES):
        half = c % 2
        h0 = half * HS
        lo, hi = h0 - 3, h0 + HS + 5
        s, e = max(lo, 0), min(hi, H)
        rowmask[c, :, s - lo:e - lo] = 1.0

    return {
        "ones1": np.ones((NCORES, P), f16),
        "kyc": np.tile(kyc, (NCORES, 1, 1)),
        "kxc": np.tile(kxc, (NCORES, 1, 1)),
        "sidx": np.tile(sidx, (NCORES, 1)),
        "rowmask": rowmask.reshape(NCORES * P, RV),
    }


def _build():
    if "fn" in _CACHE:
        return
    nc = bacc.Bacc("TRN2", target_bir_lowering=False, debug=False, num_devices=NCORES)
    io = [nc.dram_tensor(nm, sh, dt, kind=kd).ap() for nm, sh, dt, kd in SPECS]
    with tile.TileContext(nc) as tc:
        _emit(tc, nc, io)
    nc.compile()

    bass2jax.install_neuronx_cc_hook()
    partition_name = nc.partition_id_tensor.name if nc.partition_id_tensor else None
    in_names, out_names, out_avals = [], [], []
    for alloc in nc.m.functions[0].allocations:
        if not isinstance(alloc, mybir.MemoryLocationSet):
            continue
        name = alloc.memorylocations[0].name
        if alloc.kind == "ExternalInput":
            if name != partition_name:
                in_names.append(name)
        elif alloc.kind == "ExternalOutput":
            shape = tuple(alloc.tensor_shape)
            dtype = mybir.dt.np(alloc.dtype)
            out_names.append(name)
            out_avals.append(jax.core.ShapedArray(shape, dtype))
    assert nc.dbg_addr is None, "built with debug=False"
    n_params = len(in_names)
    all_names = list(in_names) + out_names
    if partition_name is not None:
        all_names.append(partition_name)

    def _body(*args):
        operands = list(args)
        if partition_name is not None:
            operands.append(bass2jax.partition_id_tensor())
        return tuple(bass2jax._bass_exec_p.bind(
            *operands,
            out_avals=tuple(out_avals),
            in_names=tuple(all_names),
            out_names=tuple(out_names),
            lowering_input_output_aliases=(),
            sim_require_finite=True,
            sim_require_nnan=True,
            nc=nc,
        ))

    devices = jax.devices()[:NCORES]
    mesh = Mesh(np.asarray(devices), ("core",))
    n_outs = len(out_names)
    fn = jax.jit(
        shard_map(_body, mesh=mesh,
                  in_specs=(PartitionSpec("core"),) * (n_params + n_outs),
                  out_specs=(PartitionSpec("core"),) * n_outs,
                  check_rep=False),
        keep_unused=True,
    )
    shd = NamedSharding(mesh, PartitionSpec("core"))
    # no donation: this kernel writes every output element, so the zero
    # "output operand" arrays are never consumed -- upload once, reuse forever
    zeros_host = [np.zeros((NCORES * a.shape[0], *a.shape[1:]), a.dtype) for a in out_avals]
    zeros_dev = [jax.device_put(z, shd) for z in zeros_host]
    consts_host = _const_globals()
    consts_dev = {k: jax.device_put(v, shd) for k, v in consts_host.items()}

    _CACHE.update(nc=nc, fn=fn, shd=shd, in_param_names=in_names,
                  out_names=out_names, out_avals=out_avals,
                  zeros_dev=zeros_dev, zeros_host=zeros_host,
                  consts_dev=consts_dev, consts_host=consts_host)


def _prep_globals(inputs):
    """Input-dependent global (8-core concatenated) arrays, keyed by tensor name."""
    f32, f16 = np.float32, np.float16
    conv_w = np.asarray(inputs["conv_w"], f32)[:, :, 0, 0]       # [co, ci]
    value_w = np.asarray(inputs["value_w"], f32)                  # [co, ci]
    offset_w = np.asarray(inputs["offset_w"], f32)                # [112, ci]
    out_w = np.asarray(inputs["out_w"], f32)                      # [co, ci]

    cw = conv_w.T.reshape(2, P, 256).transpose(1, 0, 2).astype(f16)  # [ci_p, ci_chunk, co]
    s1 = (np.asarray(inputs["bn1_gamma"], f32)
          / np.sqrt(np.asarray(inputs["bn1_var"], f32) + EPS))
    b1 = np.asarray(inputs["bn1_beta"], f32) - np.asarray(inputs["bn1_mean"], f32) * s1
    bn1s = s1.reshape(2, P).T.copy()                              # [p, co_chunk]
    bn1b = b1.reshape(2, P).T.copy()

    # permuted offset rows: [y(g,k) 36 | x(g,k) 36 | mask(g,k) 36]
    perm = np.empty(108, np.int64)
    for g in range(G):
        for k in range(K):
            perm[g * K + k] = g * 27 + 2 * k + 1
            perm[36 + g * K + k] = g * 27 + 2 * k
            perm[72 + g * K + k] = g * 27 + 18 + k
    ow_p = offset_w[perm]                                         # [108, ci]
    ob_p = np.asarray(inputs["offset_b"], f32)[perm]
    wvo_full = np.concatenate([value_w.T, ow_p.T, np.zeros((256, 4), f32)], axis=1)
    wvo = wvo_full.reshape(2, P, 368).transpose(1, 0, 2).astype(f16)
    brow = np.concatenate([np.asarray(inputs["value_b"], f32), ob_p,
                           np.zeros(4, f32)]).reshape(1, 368).astype(f16)

    owT = np.empty((P, 2, 2, P), f16)
    for ci in range(2):
        for co in range(2):
            owT[:, ci, co, :] = out_w[co * P:(co + 1) * P, ci * P:(ci + 1) * P].T
    s2 = (np.asarray(inputs["bn2_gamma"], f32)
          / np.sqrt(np.asarray(inputs["bn2_var"], f32) + EPS))
    b2 = np.asarray(inputs["bn2_beta"], f32) - np.asarray(inputs["bn2_mean"], f32) * s2
    bn2s = s2.reshape(2, P).T.copy()
    bn2b = b2.reshape(2, P).T.copy()

    x = np.asarray(inputs["x"], f32)
    xs = np.zeros((NCORES, 2, P, RV, W), f16)
    for c in range(NCORES):
        n, half = c // 2, c % 2
        h0 = half * HS
        lo, hi = h0 - 3, h0 + HS + 5                              # 72 rows
        s, e = max(lo, 0), min(hi, H)
        for ci in range(2):
            xs[c, ci, :, s - lo:e - lo, :] = x[n, ci * P:(ci + 1) * P, s:e, :]

    def rep(a):
        return np.broadcast_to(a[None], (NCORES, *a.shape)).reshape(
            NCORES * a.shape[0], *a.shape[1:])

    return {
        "x_sh": xs.reshape(NCORES * 2, P, RV * W),
        "cw": rep(cw), "bn1s": rep(bn1s), "bn1b": rep(bn1b),
        "wvo": rep(wvo), "brow": rep(brow), "owT": rep(owT),
        "bn2s": rep(bn2s), "bn2b": rep(bn2b),
    }


def _assemble(out_g):
    """(8*2, P, HS*W) fp16 -> (N, C, H, W) fp32."""
    g = np.asarray(out_g).reshape(N, 2, 2, P, HS, W)              # [n, half, co, p, h, w]
    return g.transpose(0, 2, 3, 1, 4, 5).reshape(N, C, H, W).astype(np.float32)


def _run_fast(glb):
    args = [_CACHE["consts_dev"][nm] if nm in _CACHE["consts_dev"]
            else jax.device_put(glb[nm], _CACHE["shd"])
            for nm in _CACHE["in_param_names"]]
    outs = _CACHE["fn"](*args, *_CACHE["zeros_dev"])
    return np.asarray(outs[0])


def _run_slow(glb):
    """Fallback: stock per-call runner."""
    full = dict(glb)
    full.update(_CACHE["consts_host"])
    in_maps = []
    for c in range(NCORES):
        m = {}
        for nm, sh, _, kd in SPECS:
            if kd != "ExternalInput":
                continue
            arr = full[nm]
            d0 = sh[0]
            m[nm] = np.ascontiguousarray(arr[c * d0:(c + 1) * d0])
        in_maps.append(m)
    res = bass_utils.run_bass_kernel_spmd(_CACHE["nc"], in_maps,
                                          core_ids=list(range(NCORES)))
    return np.concatenate([res.results[c]["out"] for c in range(NCORES)], axis=0)


def kernel(**inputs):
    _build()
    glb = _prep_globals(inputs)
    try:
        out_g = _run_fast(glb)
    except Exception:
        out_g = _run_slow(glb)
    return _assemble(out_g)


# revision 13
# speedup vs baseline: 2836.4804x; 1.0402x over previous
"""DCNv4 block (conv1x1+BN+SiLU -> value/offset proj -> deformable agg -> out proj+BN+SiLU)
on 8 trn2 NeuronCores. Data-parallel over (sample, row-half) with 3/4-row halos.

Deformable aggregation strategy: all 36 bilinear corners per (token, group) land in a
fixed 8x7 patch around the token (offsets are small). Patch weights are built densely
with hat functions (no floor/gather), scattered into a dense sparse-matrix row block
S^T[token, (rho, w')] via gpsimd local_scatter with a constant shear index table,
DMA-transposed to S[(w'), rho, token], and contracted against token-major values on
the PE: dcn^T[c, t] = sum_rho v^T[w', row, c]^T @ S[w', rho, t].

Execution path: the axon tunnel (~20-40 MB/s) dominates end-to-end time, so the
runner (a) ships x and the matmul weights in fp16 and returns the output in fp16,
(b) builds the shard_map jit ONCE and reuses it (the stock run_bass_kernel_spmd
re-traces and re-compiles per call), (c) keeps the input-independent geometry
constants and the zero output operands resident on device (no donation -- this
kernel writes every output element, so fresh uninit result buffers are fine).
"""
import numpy as np

import jax
from jax.experimental.shard_map import shard_map
from jax.sharding import Mesh, NamedSharding, PartitionSpec

from concourse import bass, mybir, tile, bacc, bass_utils, bass2jax

# ---- problem constants (hardcoded; kernel.py must be self-contained) ----
N, C, H, W = 4, 256, 128, 128
G, KS, K = 4, 3, 9
Cg = C // G
PAD_OFF = 112
EPS = 1e-5
NCORES = 8
HS = H // 2                    # interior rows per core
RV = 72                        # v rows per core: 3 halo top + 64 + 4 halo bottom + 1 pad
RHO, DEL = 8, 7                # patch extent (rows x cols)
NSLOT = RHO * DEL              # 56
TAU = RHO * W                  # 1024
NBLK = RV // 4                 # stage-1/2 row blocks of 4

fp32 = mybir.dt.float32
fp16 = mybir.dt.float16
i16 = mybir.dt.int16
AF = mybir.ActivationFunctionType
ALU = mybir.AluOpType
P = 128

# input-independent inputs (geometry tables): uploaded once, cached on device
CONST_NAMES = ("ones1", "kyx", "sidx", "rowmask")


def _emit(tc, nc, io):
    x_sh, cw, bn1s, bn1b, wvo, brow, ones1, kyx, sidx, owT, bn2s, bn2b, rowmask, out_d = io

    with tc.tile_pool(name="const", bufs=1) as cp, \
         tc.tile_pool(name="big", bufs=1) as bp, \
         tc.tile_pool(name="s12", bufs=2) as p12, \
         tc.tile_pool(name="s12ps", bufs=2, space="PSUM") as ps12, \
         tc.tile_pool(name="s3a", bufs=4) as p3a, \
         tc.tile_pool(name="s3b", bufs=3) as p3b, \
         tc.tile_pool(name="s3c", bufs=2) as p3c, \
         tc.tile_pool(name="s3ps", bufs=2, space="PSUM") as ps3:

        # ---- load constants ----
        cw_sb = cp.tile([P, 2, 256], fp16)
        wvo_sb = cp.tile([P, 2, 368], fp16)
        brow_sb = cp.tile([1, 368], fp16)
        ones_sb = cp.tile([1, P], fp16)
        bn1s_sb = cp.tile([P, 2], fp32)
        bn1b_sb = cp.tile([P, 2], fp32)
        kyx_sb = cp.tile([P, 72, RHO], fp16)
        sidx_sb = cp.tile([P, NSLOT], i16)
        owT_sb = cp.tile([P, 2, 2, P], fp16)
        bn2s_sb = cp.tile([P, 2], fp32)
        bn2b_sb = cp.tile([P, 2], fp32)
        rmask_sb = cp.tile([P, RV], fp16)
        for sb, dr in ((cw_sb, cw), (wvo_sb, wvo), (brow_sb, brow), (ones_sb, ones1),
                       (bn1s_sb, bn1s), (bn1b_sb, bn1b), (kyx_sb, kyx),
                       (sidx_sb, sidx), (owT_sb, owT), (bn2s_sb, bn2s), (bn2b_sb, bn2b),
                       (rmask_sb, rowmask)):
            nc.sync.dma_start(sb[:], dr)

        v_sb = bp.tile([P, RV, 256], fp16)
        om_sb = bp.tile([P, HS, 108], fp16)
        x_sb = bp.tile([P, 2, RV * W], fp16)       # whole x shard SBUF-resident
        CH = 16                                     # output rows per staged DMA chunk
        out_c = bp.tile([P, 2, 2, CH * W], fp16)    # [p, slot, co, chunk]
        for ci in range(2):
            nc.sync.dma_start(x_sb[:, ci, :], x_sh[ci])

        # ================= stage 1+2: conv+BN+SiLU, value/offset proj =================
        for blk in range(NBLK):
            y_sb = p12.tile([P, 2, 512], fp16, tag="y")
            for co in range(2):
                y_ps = ps12.tile([P, 512], fp32, space="PSUM", tag="yps")
                for ci in range(2):
                    nc.tensor.matmul(out=y_ps[:], lhsT=cw_sb[:, ci, co * P:(co + 1) * P],
                                     rhs=x_sb[:, ci, blk * 512:(blk + 1) * 512],
                                     start=(ci == 0), stop=(ci == 1))
                nc.scalar.activation(y_sb[:, co, :], y_ps[:], AF.Silu,
                                     scale=bn1s_sb[:, co:co + 1], bias=bn1b_sb[:, co:co + 1])
            for r4 in range(4):
                rr = blk * 4 + r4
                p_ps = ps12.tile([P, 368], fp32, space="PSUM", tag="pps")
                for ci in range(2):
                    nc.tensor.matmul(out=p_ps[:], lhsT=y_sb[:, ci, r4 * P:(r4 + 1) * P],
                                     rhs=wvo_sb[:, ci, :], start=(ci == 0), stop=False)
                nc.tensor.matmul(out=p_ps[:], lhsT=ones_sb[:], rhs=brow_sb[:],
                                 start=False, stop=True)
                nc.scalar.activation(v_sb[:, rr, :], p_ps[:, 0:256], AF.Copy)
                if 3 <= rr < 3 + HS:
                    nc.scalar.activation(om_sb[:, rr - 3, :], p_ps[:, 256:364], AF.Copy)

        # zero out-of-image halo rows of v (per-core row mask)
        nc.vector.tensor_tensor(out=v_sb[:], in0=v_sb[:],
                                in1=rmask_sb[:].unsqueeze(2).to_broadcast([P, RV, 256]),
                                op=ALU.mult)

        # ================= stage 3: deformable aggregation per output row =============
        dcn2 = None
        for h in range(HS):
            # hat weights for both axes in one shot: u = 1 - |kyx - off|, clamped at 0
            u = p3a.tile([P, 72, RHO], fp16, tag="u")
            nc.vector.tensor_tensor(out=u[:], in0=kyx_sb[:],
                                    in1=om_sb[:, h, 0:72].unsqueeze(2).to_broadcast([P, 72, RHO]),
                                    op=ALU.subtract)
            nc.scalar.activation(u[:], u[:], AF.Abs)
            nc.scalar.activation(u[:], u[:], AF.Relu, scale=-1.0, bias=1.0)
            aym = p3a.tile([P, 36, RHO], fp16, tag="aym")
            nc.vector.tensor_tensor(out=aym[:], in0=u[:, 0:36, :],
                                    in1=om_sb[:, h, 72:108].unsqueeze(2).to_broadcast([P, 36, RHO]),
                                    op=ALU.mult)

            # prod memory layout [g][rho][del][k]; write iterated as (g,k,rho,del)
            prod = p3a.tile([P, G, RHO, DEL, K], fp16, tag="prod")
            for g in range(G):
                pv = prod[:, g].rearrange("p r d k -> p k r d")
                nc.vector.tensor_tensor(
                    out=pv,
                    in0=aym[:, g * K:(g + 1) * K, :].unsqueeze(3).to_broadcast([P, K, RHO, DEL]),
                    in1=u[:, 36 + g * K:36 + (g + 1) * K, 0:DEL].unsqueeze(2).to_broadcast([P, K, RHO, DEL]),
                    op=ALU.mult)
            P16t = p3a.tile([P, G * NSLOT], fp16, tag="P16")
            with nc.allow_low_precision(reason="sum of <=4 hat-window terms in [0,1]*mask"):
                nc.vector.tensor_reduce(out=P16t[:],
                                        in_=prod[:].rearrange("p g r d k -> p (g r d) k"),
                                        axis=mybir.AxisListType.X, op=ALU.add)

            ST = p3b.tile([P, G, TAU], fp16, tag="ST")
            for g in range(G):
                nc.gpsimd.local_scatter(ST[:, g, :], P16t[:, g * NSLOT:(g + 1) * NSLOT],
                                        sidx_sb[:], channels=P,
                                        num_elems=TAU, num_idxs=NSLOT)
            S = p3b.tile([W, G, RHO, P], fp16, tag="S")
            nc.sync.dma_start_transpose(out=S[:], in_=ST[:])
            dc = ps3.tile([P, 2, P], fp32, space="PSUM", tag="dc")
            for g in range(G):
                po = (g % 2) * 64
                for rho in range(RHO):
                    nc.tensor.matmul(out=dc[po:po + 64, g // 2, :],
                                     lhsT=v_sb[:, h + rho, g * Cg:(g + 1) * Cg],
                                     rhs=S[:, g, rho, :], start=(rho == 0), stop=(rho == 7))
            hp = h % 2
            if hp == 0:
                dcn2 = p3c.tile([P, 2, 2, P], fp16, tag="dcn2")   # [p, ci_half, h_pair, t]
            for half in range(2):
                nc.scalar.activation(dcn2[:, half, hp, :], dc[:, half, :], AF.Copy)

            if hp == 1:
                o_ps = ps3.tile([P, 2, 2 * P], fp32, space="PSUM", tag="ops")
                for co in range(2):
                    for ci in range(2):
                        nc.tensor.matmul(out=o_ps[:, co, :], lhsT=owT_sb[:, ci, co, :],
                                         rhs=dcn2[:, ci].rearrange("p a b -> p (a b)"),
                                         start=(ci == 0), stop=(ci == 1))
                slot, hh = (h // CH) % 2, h % CH
                for co in range(2):
                    nc.scalar.activation(out_c[:, slot, co, (hh - 1) * W:(hh + 1) * W],
                                         o_ps[:, co, :], AF.Silu,
                                         scale=bn2s_sb[:, co:co + 1], bias=bn2b_sb[:, co:co + 1])
                    if hh == CH - 1:
                        nc.sync.dma_start(out_d[co, :, (h - CH + 1) * W:(h + 1) * W],
                                          out_c[:, slot, co, :])


_CACHE = {}

SPECS = [
    ("x_sh", [2, P, RV * W], fp16, "ExternalInput"),
    ("cw", [P, 2, 256], fp16, "ExternalInput"),
    ("bn1s", [P, 2], fp32, "ExternalInput"),
    ("bn1b", [P, 2], fp32, "ExternalInput"),
    ("wvo", [P, 2, 368], fp16, "ExternalInput"),
    ("brow", [1, 368], fp16, "ExternalInput"),
    ("ones1", [1, P], fp16, "ExternalInput"),
    ("kyx", [P, 72, RHO], fp16, "ExternalInput"),
    ("sidx", [P, NSLOT], i16, "ExternalInput"),
    ("owT", [P, 2, 2, P], fp16, "ExternalInput"),
    ("bn2s", [P, 2], fp32, "ExternalInput"),
    ("bn2b", [P, 2], fp32, "ExternalInput"),
    ("rowmask", [P, RV], fp16, "ExternalInput"),
    ("out", [2, P, HS * W], fp16, "ExternalOutput"),
]


def _const_globals():
    """Input-independent geometry tables, pre-concatenated over the 8 cores."""
    f16 = np.float16
    ks = np.arange(K)
    ik, jk = ks // 3, ks % 3
    rho = np.arange(RHO)
    dl = np.arange(DEL)
    kyc1 = rho[None, :] - 3 - (ik[:, None] - 1)                   # [k, rho]
    kxc1 = dl[None, :] - 3 - (jk[:, None] - 1)                    # [k, del]
    kyx1 = np.full((72, RHO), -1000.0, np.float32)                # pad col -> hat weight 0
    kyx1[0:36, :] = np.tile(kyc1, (G, 1))
    kyx1[36:72, 0:DEL] = np.tile(kxc1, (G, 1))
    kyx = np.broadcast_to(kyx1[None], (P, 72, RHO)).astype(f16)

    sidx = np.empty((P, NSLOT), np.int16)
    for t in range(P):
        for r in range(RHO):
            for d in range(DEL):
                w = t + d - 3
                sidx[t, r * DEL + d] = r * W + w if 0 <= w < W else -1

    rowmask = np.zeros((NCORES, P, RV), f16)
    for c in range(NCORES):
        half = c % 2
        h0 = half * HS
        lo, hi = h0 - 3, h0 + HS + 5
        s, e = max(lo, 0), min(hi, H)
        rowmask[c, :, s - lo:e - lo] = 1.0

    return {
        "ones1": np.ones((NCORES, P), f16),
        "kyx": np.tile(kyx, (NCORES, 1, 1)),
        "sidx": np.tile(sidx, (NCORES, 1)),
        "rowmask": rowmask.reshape(NCORES * P, RV),
    }


def _build():
    if "fn" in _CACHE:
        return
    nc = bacc.Bacc("TRN2", target_bir_lowering=False, debug=False, num_devices=NCORES)
    io = [nc.dram_tensor(nm, sh, dt, kind=kd).ap() for nm, sh, dt, kd in SPECS]
    with tile.TileContext(nc) as tc:
        _emit(tc, nc, io)
    nc.compile()

    bass2jax.install_neuronx_cc_hook()
    partition_name = nc.partition_id_tensor.name if nc.partition_id_tensor else None
    in_names, out_names, out_avals = [], [], []
    for alloc in nc.m.functions[0].allocations:
        if not isinstance(alloc, mybir.MemoryLocationSet):
            continue
        name = alloc.memorylocations[0].name
        if alloc.kind == "ExternalInput":
            if name != partition_name:
                in_names.append(name)
        elif alloc.kind == "ExternalOutput":
            shape = tuple(alloc.tensor_shape)
            dtype = mybir.dt.np(alloc.dtype)
            out_names.append(name)
            out_avals.append(jax.core.ShapedArray(shape, dtype))
    assert nc.dbg_addr is None, "built with debug=False"
    n_params = len(in_names)
    all_names = list(in_names) + out_names
    if partition_name is not None:
        all_names.append(partition_name)

    def _body(*args):
        operands = list(args)
        if partition_name is not None:
            operands.append(bass2jax.partition_id_tensor())
        return tuple(bass2jax._bass_exec_p.bind(
            *operands,
            out_avals=tuple(out_avals),
            in_names=tuple(all_names),
            out_names=tuple(out_names),
            lowering_input_output_aliases=(),
            sim_require_finite=True,
            sim_require_nnan=True,
            nc=nc,
        ))

    devices = jax.devices()[:NCORES]
    mesh = Mesh(np.asarray(devices), ("core",))
    n_outs = len(out_names)
    fn = jax.jit(
        shard_map(_body, mesh=mesh,
                  in_specs=(PartitionSpec("core"),) * (n_params + n_outs),
                  out_specs=(PartitionSpec("core"),) * n_outs,
                  check_rep=False),
        keep_unused=True,
    )
    shd = NamedSharding(mesh, PartitionSpec("core"))
    # no donation: this kernel writes every output element, so the zero
    # "output operand" arrays are never consumed -- upload once, reuse forever
    zeros_host = [np.zeros((NCORES * a.shape[0], *a.shape[1:]), a.dtype) for a in out_avals]
    zeros_dev = [jax.device_put(z, shd) for z in zeros_host]
    consts_host = _const_globals()
    consts_dev = {k: jax.device_put(v, shd) for k, v in consts_host.items()}

    _CACHE.update(nc=nc, fn=fn, shd=shd, in_param_names=in_names,
                  out_names=out_names, out_avals=out_avals,
                  zeros_dev=zeros_dev, zeros_host=zeros_host,
                  consts_dev=consts_dev, consts_host=consts_host)


def _prep_globals(inputs):
    """Input-dependent global (8-core concatenated) arrays, keyed by tensor name."""
    f32, f16 = np.float32, np.float16
    conv_w = np.asarray(inputs["conv_w"], f32)[:, :, 0, 0]       # [co, ci]
    value_w = np.asarray(inputs["value_w"], f32)                  # [co, ci]
    offset_w = np.asarray(inputs["offset_w"], f32)                # [112, ci]
    out_w = np.asarray(inputs["out_w"], f32)                      # [co, ci]

    cw = conv_w.T.reshape(2, P, 256).transpose(1, 0, 2).astype(f16)  # [ci_p, ci_chunk, co]
    s1 = (np.asarray(inputs["bn1_gamma"], f32)
          / np.sqrt(np.asarray(inputs["bn1_var"], f32) + EPS))
    b1 = np.asarray(inputs["bn1_beta"], f32) - np.asarray(inputs["bn1_mean"], f32) * s1
    bn1s = s1.reshape(2, P).T.copy()                              # [p, co_chunk]
    bn1b = b1.reshape(2, P).T.copy()

    # permuted offset rows: [y(g,k) 36 | x(g,k) 36 | mask(g,k) 36]
    perm = np.empty(108, np.int64)
    for g in range(G):
        for k in range(K):
            perm[g * K + k] = g * 27 + 2 * k + 1
            perm[36 + g * K + k] = g * 27 + 2 * k
            perm[72 + g * K + k] = g * 27 + 18 + k
    ow_p = offset_w[perm]                                         # [108, ci]
    ob_p = np.asarray(inputs["offset_b"], f32)[perm]
    wvo_full = np.concatenate([value_w.T, ow_p.T, np.zeros((256, 4), f32)], axis=1)
    wvo = wvo_full.reshape(2, P, 368).transpose(1, 0, 2).astype(f16)
    brow = np.concatenate([np.asarray(inputs["value_b"], f32), ob_p,
                           np.zeros(4, f32)]).reshape(1, 368).astype(f16)

    owT = np.empty((P, 2, 2, P), f16)
    for ci in range(2):
        for co in range(2):
            owT[:, ci, co, :] = out_w[co * P:(co + 1) * P, ci * P:(ci + 1) * P].T
    s2 = (np.asarray(inputs["bn2_gamma"], f32)
          / np.sqrt(np.asarray(inputs["bn2_var"], f32) + EPS))
    b2 = np.asarray(inputs["bn2_beta"], f32) - np.asarray(inputs["bn2_mean"], f32) * s2
    bn2s = s2.reshape(2, P).T.copy()
    bn2b = b2.reshape(2, P).T.copy()

    x = np.asarray(inputs["x"], f32)
    xs = np.zeros((NCORES, 2, P, RV, W), f16)
    for c in range(NCORES):
        n, half = c // 2, c % 2
        h0 = half * HS
        lo, hi = h0 - 3, h0 + HS + 5                              # 72 rows
        s, e = max(lo, 0), min(hi, H)
        for ci in range(2):
            xs[c, ci, :, s - lo:e - lo, :] = x[n, ci * P:(ci + 1) * P, s:e, :]

    def rep(a):
        return np.broadcast_to(a[None], (NCORES, *a.shape)).reshape(
            NCORES * a.shape[0], *a.shape[1:])

    return {
        "x_sh": xs.reshape(NCORES * 2, P, RV * W),
        "cw": rep(cw), "bn1s": rep(bn1s), "bn1b": rep(bn1b),
        "wvo": rep(wvo), "brow": rep(brow), "owT": rep(owT),
        "bn2s": rep(bn2s), "bn2b": rep(bn2b),
    }


def _assemble(out_g):
    """(8*2, P, HS*W) fp16 -> (N, C, H, W) fp32."""
    g = np.asarray(out_g).reshape(N, 2, 2, P, HS, W)              # [n, half, co, p, h, w]
    return g.transpose(0, 2, 3, 1, 4, 5).reshape(N, C, H, W).astype(np.float32)


def _run_fast(glb):
    args = [_CACHE["consts_dev"][nm] if nm in _CACHE["consts_dev"]
            else jax.device_put(glb[nm], _CACHE["shd"])
            for nm in _CACHE["in_param_names"]]
    outs = _CACHE["fn"](*args, *_CACHE["zeros_dev"])
    return np.asarray(outs[0])


def _run_slow(glb):
    """Fallback: stock per-call runner."""
    full = dict(glb)
    full.update(_CACHE["consts_host"])
    in_maps = []
    for c in range(NCORES):
        m = {}
        for nm, sh, _, kd in SPECS:
            if kd != "ExternalInput":
                continue
            arr = full[nm]
            d0 = sh[0]
            m[nm] = np.ascontiguousarray(arr[c * d0:(c + 1) * d0])
        in_maps.append(m)
    res = bass_utils.run_bass_kernel_spmd(_CACHE["nc"], in_maps,
                                          core_ids=list(range(NCORES)))
    return np.concatenate([res.results[c]["out"] for c in range(NCORES)], axis=0)


def kernel(**inputs):
    _build()
    glb = _prep_globals(inputs)
    try:
        out_g = _run_fast(glb)
    except Exception:
        out_g = _run_slow(glb)
    return _assemble(out_g)
